# revision 15
# baseline (speedup 1.0000x reference)
"""GraphSAGE (2-layer, mean aggregation) on 8 Trainium2 NeuronCores.

Strategy (v5):
  - Nodes sharded across 8 cores by destination; per-core permutation
    balances per-64-row-block in-edge loads (layer1 / layer2-g1 / layer2-g2).
  - Features gathered as fp8 e4m3 PAIRS (row p = nodes 2p, 2p+1, 256 B);
    pair indices fit int16. dma_gather descriptor generation on the Q7
    cores is the hard throughput wall (~1.8 ns/edge), so every other
    engine's work is kept below it:
      * edges of each (block, stream) are sorted by src parity, so almost
        all 128-edge chunks are parity-pure: ONE bf16 one-hot + ONE fp8
        matmul per chunk; only boundary chunks need the dual form.
      * dst blocks are 64 wide, halving one-hot Vector work.
  - Layer-1 h rows are computed per block pair inside the aggregation
    loop, stored to hsh in fp8. The h exchange is TWO AllGathers: rows of
    blocks 0..49 (triggered mid-layer-1) and the rest. Layer-2 edges are
    split by source half (g1 from hfullA, g2 from hfullB) and aggregated
    in two passes with an SBUF accumulator, so g1 gathers and the second
    collective overlap.
"""

import math
from contextlib import ExitStack

import numpy as np
import ml_dtypes

import concourse.bass as bass
import concourse.bacc as bacc
import concourse.mybir as mybir
import concourse.tile as tile
from concourse import bass_utils

P = 128
BW = 64                                # dst block width
N_NODES = 50000
D_IN = 128
D_HID = 128
D_OUT = 40
N_CORES = 8
ROWS_PER = N_NODES // N_CORES          # 6250
NBLK = math.ceil(ROWS_PER / BW)        # 98
NBLK1 = 50                             # blocks in half 1
H1_ROWS = NBLK1 * BW                   # 3200
H2_ROWS = ROWS_PER - H1_ROWS           # 3050
GRP = 32                               # chunks per dma_gather call
GBUFS = 6
OBUFS = 4
NQ = 4

BF16 = ml_dtypes.bfloat16
FP8 = ml_dtypes.float8_e4m3


def _wrap_idxs(idx_flat):
    n = idx_flat.shape[0]
    assert n % 16 == 0
    w = idx_flat.reshape(n // 16, 16).T.astype(np.int16)
    return np.tile(w, (8, 1))


def _greedy_assign(items, loads, nbins, caps, targets):
    order = np.argsort(-loads.sum(axis=1), kind="stable")
    binloads = np.zeros((nbins, loads.shape[1]))
    cnt = np.zeros(nbins, np.int64)
    bins = [[] for _ in range(nbins)]
    tgt = np.maximum(targets, 1e-9)
    for i in order:
        cost = ((binloads + loads[i][None, :]) / tgt).max(axis=1)
        cost[cnt >= caps] = np.inf
        b = int(np.argmin(cost))
        bins[b].append(items[i])
        binloads[b] += loads[i]
        cnt[b] += 1
    return bins


def preprocess(edge_index):
    src0 = np.asarray(edge_index[0], dtype=np.int64)
    dst0 = np.asarray(edge_index[1], dtype=np.int64)
    deg_in = np.bincount(dst0, minlength=N_NODES)
    deg_out = np.bincount(src0, minlength=N_NODES)

    # step 1: split each core's nodes into half1 (3200) / half2 (3050)
    half1 = np.zeros(N_NODES, bool)
    for k in range(N_CORES):
        nodes = np.arange(k * ROWS_PER, (k + 1) * ROWS_PER)
        loads = np.stack([deg_in[nodes], deg_out[nodes]], axis=1).astype(
            np.float64)
        tot = loads.sum(axis=0)
        caps = np.array([H1_ROWS, H2_ROWS])
        targets = np.stack([tot * H1_ROWS / ROWS_PER, tot * H2_ROWS / ROWS_PER])
        bins = _greedy_assign(nodes, loads, 2, caps, targets)
        half1[np.asarray(bins[0], np.int64)] = True

    g1_edge = half1[src0]
    in_g1 = np.bincount(dst0[g1_edge], minlength=N_NODES)
    in_g2 = deg_in - in_g1

    # step 2: assign nodes to 64-row blocks within each half
    perm = np.empty(N_NODES, np.int64)
    for k in range(N_CORES):
        base = k * ROWS_PER
        nodes = np.arange(base, base + ROWS_PER)
        for half, nb, r0 in ((True, NBLK1, 0), (False, NBLK - NBLK1, H1_ROWS)):
            hn = nodes[half1[nodes] == half]
            loads = np.stack(
                [deg_in[hn], in_g1[hn], in_g2[hn]], axis=1).astype(np.float64)
            caps = np.full(nb, BW, np.int64)
            caps[-1] = hn.shape[0] - (nb - 1) * BW
            tot = loads.sum(axis=0)
            targets = caps[:, None] * (tot[None, :] / hn.shape[0])
            bins = _greedy_assign(hn, loads, nb, caps, targets)
            off = base + r0
            for b in range(nb):
                ids = np.asarray(bins[b], np.int64)
                perm[off : off + ids.shape[0]] = ids
                off += ids.shape[0]

    slot_of = np.empty(N_NODES, np.int64)
    slot_of[perm] = np.arange(N_NODES)
    src = slot_of[src0]
    dst = slot_of[dst0]
    counts = np.bincount(dst, minlength=N_NODES)
    inv_deg = (1.0 / np.maximum(counts, 1)).astype(np.float32)

    # gather-table pair rows per stream
    s_core = src // ROWS_PER
    s_loc = src % ROWS_PER
    is_g1 = s_loc < H1_ROWS
    row_l1 = src // 2
    row_g1 = s_core * (H1_ROWS // 2) + s_loc // 2
    row_g2 = s_core * (H2_ROWS // 2) + (s_loc - H1_ROWS) // 2

    order = np.argsort(dst, kind="stable")
    d_s = dst[order]
    par_s = (src % 2)[order]
    g1_s = is_g1[order]
    rows_s = {"l1": row_l1[order], "g1": row_g1[order], "g2": row_g2[order]}

    # per (core, block, stream): edges sorted evens-first
    seg = {}
    counts_bs = {s: np.zeros(NBLK, np.int64) for s in ("l1", "g1", "g2")}
    ne = {s: np.zeros((N_CORES, NBLK), np.int64) for s in ("l1", "g1", "g2")}
    for k in range(N_CORES):
        base = k * ROWS_PER
        for b in range(NBLK):
            r0 = base + b * BW
            r1 = min(base + ROWS_PER, r0 + BW)
            e0 = np.searchsorted(d_s, r0, side="left")
            e1 = np.searchsorted(d_s, r1, side="left")
            sl = slice(e0, e1)
            sel = {"l1": np.ones(e1 - e0, bool), "g1": g1_s[sl],
                   "g2": ~g1_s[sl]}
            for s in ("l1", "g1", "g2"):
                m = sel[s]
                rr = rows_s[s][sl][m]
                dd = (d_s[sl][m] - r0)
                pp = par_s[sl][m]
                ev = pp == 0
                rr = np.concatenate([rr[ev], rr[~ev]])
                dd = np.concatenate([dd[ev], dd[~ev]])
                seg[(k, b, s)] = (rr, dd, int(ev.sum()))
                ne[s][k, b] = int(ev.sum())
                counts_bs[s][b] = max(counts_bs[s][b],
                                      (rr.shape[0] + P - 1) // P)

    offs, labels, Cs = {}, {}, {}
    for s in ("l1", "g1", "g2"):
        off = np.zeros(NBLK + 1, np.int64)
        off[1:] = np.cumsum(counts_bs[s])
        offs[s] = off
        C = int(off[-1])
        Cs[s] = C
        lab = np.zeros(C, np.int8)
        for b in range(NBLK):
            tmin = int(ne[s][:, b].min())
            tmax = int(ne[s][:, b].max())
            for c in range(int(counts_bs[s][b])):
                s0, s1 = c * P, (c + 1) * P
                if s1 <= tmin:
                    lab[off[b] + c] = 0
                elif s0 >= tmax:
                    lab[off[b] + c] = 1
                else:
                    lab[off[b] + c] = 2
        labels[s] = lab

    per_core = []
    for k in range(N_CORES):
        pc = {}
        for s in ("l1", "g1", "g2"):
            C = Cs[s]
            off = offs[s]
            lab_s = labels[s]
            idx = np.zeros((C, P), np.int16)
            dstv = np.full((C, P, 2), -1.0, np.float32)
            for b in range(NBLK):
                rr, dd, _ = seg[(k, b, s)]
                n = rr.shape[0]
                c0 = int(off[b])
                nch = int(counts_bs[s][b])
                fl_i = idx[c0 : c0 + nch].reshape(-1)
                fl_d = dstv[c0 : c0 + nch].reshape(-1, 2)
                fl_i[:n] = rr.astype(np.int16)
                labv = np.repeat(lab_s[c0 : c0 + nch], P)[:n]
                nev = seg[(k, b, s)][2]
                par = (np.arange(n) >= nev).astype(np.int64)
                lane = np.where(labv == 2, par, 0)
                fl_d[np.arange(n), lane] = dd.astype(np.float32)
            pc["idx_" + s] = _wrap_idxs(idx.reshape(-1))
            pc["dstv_" + s] = np.ascontiguousarray(
                dstv.transpose(1, 0, 2)).astype(BF16)
        pc["invdeg"] = np.tile(
            inv_deg[k * ROWS_PER : (k + 1) * ROWS_PER][None, :], (P, 1)
        ).astype(BF16)
        per_core.append(pc)

    meta = dict(
        perm=perm, offs=offs, Cs=Cs,
        counts_key=tuple(tuple(int(v) for v in counts_bs[s]) for s in
                         ("l1", "g1", "g2")),
        labels={s: tuple(int(v) for v in labels[s]) for s in
                ("l1", "g1", "g2")},
    )
    return meta, per_core


def build_graph(nc, m):
    dt = mybir.dt
    alu = mybir.AluOpType
    act = mybir.ActivationFunctionType
    Cs, offs, labels = m["Cs"], m["offs"], m["labels"]
    STR = ("l1", "g1", "g2")

    xp_d = nc.dram_tensor("xp", [N_NODES // 2, 2 * D_IN], dt.float8e4,
                          kind="ExternalInput")
    xT_d = nc.dram_tensor("xT", [P, ROWS_PER], dt.bfloat16, kind="ExternalInput")
    idx_d = {s: nc.dram_tensor(f"idx_{s}", [P, Cs[s] * 8], dt.int16,
                               kind="ExternalInput") for s in STR}
    dstv_d = {s: nc.dram_tensor(f"dstv_{s}", [P, Cs[s], 2], dt.bfloat16,
                                kind="ExternalInput") for s in STR}
    invdeg_d = nc.dram_tensor("invdeg", [P, ROWS_PER], dt.bfloat16,
                              kind="ExternalInput")
    iota_d = nc.dram_tensor("iota", [P, P], dt.bfloat16, kind="ExternalInput")
    w1l_d = nc.dram_tensor("w1lT", [P, D_HID], dt.bfloat16, kind="ExternalInput")
    w1r_d = nc.dram_tensor("w1rT", [P, D_HID], dt.bfloat16, kind="ExternalInput")
    w2l_d = nc.dram_tensor("w2lT", [P, D_OUT], dt.bfloat16, kind="ExternalInput")
    w2r_d = nc.dram_tensor("w2rT", [P, D_OUT], dt.bfloat16, kind="ExternalInput")
    b1_d = nc.dram_tensor("b1r", [1, D_HID], dt.bfloat16, kind="ExternalInput")
    b2_d = nc.dram_tensor("b2r", [1, D_OUT], dt.bfloat16, kind="ExternalInput")
    out_d = nc.dram_tensor("out", [ROWS_PER, D_OUT], dt.float32,
                           kind="ExternalOutput")

    with tile.TileContext(nc) as tc, ExitStack() as ctx:
        sb = ctx.enter_context(tc.tile_pool(name="sb", bufs=1))
        dram = ctx.enter_context(tc.tile_pool(name="dram", bufs=1, space="DRAM"))
        psA = ctx.enter_context(tc.tile_pool(name="psA", bufs=1, space="PSUM"))
        psB = ctx.enter_context(tc.tile_pool(name="psB", bufs=1, space="PSUM"))
        g_p = ctx.enter_context(tc.tile_pool(name="gp", bufs=GBUFS))
        o_p = ctx.enter_context(tc.tile_pool(name="oh", bufs=OBUFS))
        st_p = ctx.enter_context(tc.tile_pool(name="st", bufs=3))

        def load(shape, dtype, src, name):
            t = sb.tile(shape, dtype, name=name)
            nc.sync.dma_start(t[:], src[:])
            return t

        # small tiles first (they gate the first one-hot / matmul)
        iota_sb = load([P, P], dt.bfloat16, iota_d.ap(), "iota_sb")
        w1l_sb = load([P, D_HID], dt.bfloat16, w1l_d.ap(), "w1l_sb")
        w1r_sb = load([P, D_HID], dt.bfloat16, w1r_d.ap(), "w1r_sb")
        w2l_sb = load([P, D_OUT], dt.bfloat16, w2l_d.ap(), "w2l_sb")
        w2r_sb = load([P, D_OUT], dt.bfloat16, w2r_d.ap(), "w2r_sb")
        b1_sb = load([1, D_HID], dt.bfloat16, b1_d.ap(), "b1_sb")
        b2_sb = load([1, D_OUT], dt.bfloat16, b2_d.ap(), "b2_sb")

        # idx/dstv: load the first couple of gather groups' worth first
        idx_sb, dstv_sb = {}, {}
        for s in STR:
            idx_sb[s] = sb.tile([P, Cs[s] * 8], dt.int16, name=f"idx_{s}_sb")
            dstv_sb[s] = sb.tile([P, Cs[s], 2], dt.bfloat16, name=f"dstv_{s}_sb")
        pieces = [(0, 2 * GRP), (2 * GRP, 8 * GRP), (8 * GRP, 10 ** 9)]
        for s in STR:
            for a, b_ in pieces:
                a = min(a, Cs[s]); b_ = min(b_, Cs[s])
                if a >= b_:
                    continue
                nc.sync.dma_start(idx_sb[s][:, a * 8 : b_ * 8],
                                  idx_d[s].ap()[:, a * 8 : b_ * 8])
                nc.sync.dma_start(dstv_sb[s][:, a:b_, :],
                                  dstv_d[s].ap()[:, a:b_, :])
        xT_sb = load([P, ROWS_PER], dt.bfloat16, xT_d.ap(), "xT_sb")
        invdeg_sb = load([P, ROWS_PER], dt.bfloat16, invdeg_d.ap(), "invdeg_sb")

        ones_sb = sb.tile([1, 512], dt.bfloat16, name="ones_sb")
        nc.vector.memset(ones_sb[:], 1.0)

        meanT = sb.tile([P, ROWS_PER], dt.bfloat16, name="meanT")
        meanhT = sb.tile([P, ROWS_PER], dt.bfloat16, name="meanhT")
        accT = sb.tile([P, ROWS_PER], dt.bfloat16, name="accT")
        hT = sb.tile([P, ROWS_PER], dt.bfloat16, name="hT")

        hsh = dram.tile([ROWS_PER, D_IN], dt.float8e4, name="hsh")
        hfullA = dram.tile([N_CORES * H1_ROWS // 2, 2 * D_IN], dt.float8e4,
                           name="hfullA")
        hfullB = dram.tile([N_CORES * H2_ROWS // 2, 2 * D_IN], dt.float8e4,
                           name="hfullB")

        qctr = [0]
        src_ap = {"l1": xp_d.ap(), "g1": hfullA[:], "g2": hfullB[:]}
        tiles = {}

        def ensure_group(s, g):
            if (s, g) in tiles:
                return tiles[(s, g)]
            C = Cs[s]
            lab_s = labels[s]
            c0, c1 = g * GRP, min(C, (g + 1) * GRP)
            nch = c1 - c0
            n = nch * P
            t = g_p.tile([P, GRP, 2 * D_IN], dt.float8e4, tag="gt", name="gt")
            nc.gpsimd.dma_gather(
                t[:, :nch, :], src_ap[s],
                idx_sb[s][:, c0 * 8 : c1 * 8],
                n, n, 2 * D_IN, elem_step=2 * D_IN, single_packet=False,
                queue_num=qctr[0] % NQ,
            )
            qctr[0] += 1
            ot = o_p.tile([P, GRP, 2, BW], dt.bfloat16, tag="ohv", name="ohv")
            h0 = 0
            while h0 < nch:
                nl = 2 if lab_s[c0 + h0] == 2 else 1
                h1 = h0 + 1
                while (h1 < nch and h1 - h0 < GRP // 2
                       and (2 if lab_s[c0 + h1] == 2 else 1) == nl):
                    h1 += 1
                nc.vector.tensor_tensor(
                    ot[:, h0:h1, :nl, :],
                    iota_sb[:, None, None, :BW].broadcast_to(
                        [P, h1 - h0, nl, BW]),
                    dstv_sb[s][:, c0 + h0 : c0 + h1, :nl, None].broadcast_to(
                        [P, h1 - h0, nl, BW]),
                    alu.is_equal,
                )
                h0 = h1
            tiles[(s, g)] = (t, ot)
            return tiles[(s, g)]

        def accum_block(s, b, psum):
            off = offs[s]
            lab_s = labels[s]
            cs, ce = int(off[b]), int(off[b + 1])
            nmm = sum(2 if lab_s[c] == 2 else 1 for c in range(cs, ce))
            i = 0
            for c in range(cs, ce):
                gt, ot = ensure_group(s, c // GRP)
                j = c % GRP
                lab = lab_s[c]
                parities = (0, 1) if lab == 2 else (lab,)
                for o in parities:
                    lane = o if lab == 2 else 0
                    nc.tensor.matmul(
                        psum[:, :BW],
                        lhsT=gt[:, j, o * D_IN : (o + 1) * D_IN],
                        rhs=ot[:, j, lane, :],
                        start=(i == 0), stop=(i == nmm - 1),
                    )
                    i += 1
            return nmm

        def h_rows(bp):
            """h row computation for 128-column block pair bp."""
            c0 = bp * P
            bs = min(P, ROWS_PER - c0)
            ps2 = psB.tile([P, 512], dt.float32, tag="ps", name="ps_r", bufs=3)
            nc.tensor.matmul(ps2[:bs, :D_HID], lhsT=meanT[:, c0 : c0 + bs],
                             rhs=w1l_sb[:], start=True, stop=False)
            nc.tensor.matmul(ps2[:bs, :D_HID], lhsT=xT_sb[:, c0 : c0 + bs],
                             rhs=w1r_sb[:], start=False, stop=False)
            nc.tensor.matmul(ps2[:bs, :D_HID], lhsT=ones_sb[:, :bs],
                             rhs=b1_sb[:], start=False, stop=True)
            hrow = st_p.tile([P, D_HID], dt.float8e4, tag="st", name="hrow")
            nc.scalar.activation(hrow[:bs, :], ps2[:bs, :D_HID], act.Relu)
            nc.sync.dma_start(hsh[c0 : c0 + bs, :], hrow[:bs, :])

        # ================= layer 1 =================
        for b in range(NBLK):
            c0 = b * BW
            bs = min(BW, ROWS_PER - c0)
            ps = psA.tile([P, BW], dt.float32, tag="agg", name="ps_agg", bufs=4)
            accum_block("l1", b, ps)
            nc.vector.tensor_tensor(
                meanT[:, c0 : c0 + bs], ps[:, :bs],
                invdeg_sb[:, c0 : c0 + bs], alu.mult,
            )
            if b % 2 == 1:
                h_rows(b // 2)
            if b == NBLK1 - 1:
                nc.gpsimd.collective_compute(
                    "AllGather", alu.bypass,
                    replica_groups=[list(range(N_CORES))],
                    ins=[hsh[0:H1_ROWS, :].opt()], outs=[hfullA[:].opt()],
                )

        nc.gpsimd.collective_compute(
            "AllGather", alu.bypass,
            replica_groups=[list(range(N_CORES))],
            ins=[hsh[H1_ROWS:ROWS_PER, :].opt()], outs=[hfullB[:].opt()],
        )

        # col-major bf16 h panels (dense path of layer 2)
        for c0 in range(0, ROWS_PER, 512):
            w = min(512, ROWS_PER - c0)
            ps2 = psB.tile([P, 512], dt.float32, tag="ps", name="ps_d", bufs=3)
            nc.tensor.matmul(ps2[:, :w], lhsT=w1l_sb[:], rhs=meanT[:, c0 : c0 + w],
                             start=True, stop=False)
            nc.tensor.matmul(ps2[:, :w], lhsT=w1r_sb[:], rhs=xT_sb[:, c0 : c0 + w],
                             start=False, stop=False)
            nc.tensor.matmul(ps2[:, :w], lhsT=b1_sb[:], rhs=ones_sb[:, :w],
                             start=False, stop=True)
            nc.scalar.activation(hT[:, c0 : c0 + w], ps2[:, :w], act.Relu)

        # ================= layer 2 =================
        # pass 1: g1 chunks -> SBUF accumulator
        for b in range(NBLK):
            c0 = b * BW
            bs = min(BW, ROWS_PER - c0)
            ps = psA.tile([P, BW], dt.float32, tag="agg", name="ps_g1", bufs=4)
            accum_block("g1", b, ps)
            nc.vector.tensor_copy(accT[:, c0 : c0 + bs], ps[:, :bs])

        # pass 2: g2 chunks -> combine, scale; outputs per block pair
        for b in range(NBLK):
            c0 = b * BW
            bs = min(BW, ROWS_PER - c0)
            ps = psA.tile([P, BW], dt.float32, tag="agg", name="ps_g2", bufs=4)
            accum_block("g2", b, ps)
            msum = st_p.tile([P, BW], dt.bfloat16, tag="ms", name="msum")
            nc.vector.tensor_tensor(msum[:, :bs], ps[:, :bs],
                                    accT[:, c0 : c0 + bs], alu.add)
            nc.vector.tensor_tensor(meanhT[:, c0 : c0 + bs], msum[:, :bs],
                                    invdeg_sb[:, c0 : c0 + bs], alu.mult)
            if b % 2 == 1:
                bp = b // 2
                p0 = bp * P
                pbs = min(P, ROWS_PER - p0)
                ps2 = psB.tile([P, 512], dt.float32, tag="ps", name="ps_o",
                               bufs=3)
                nc.tensor.matmul(ps2[:pbs, :D_OUT], lhsT=meanhT[:, p0 : p0 + pbs],
                                 rhs=w2l_sb[:], start=True, stop=False)
                nc.tensor.matmul(ps2[:pbs, :D_OUT], lhsT=hT[:, p0 : p0 + pbs],
                                 rhs=w2r_sb[:], start=False, stop=False)
                nc.tensor.matmul(ps2[:pbs, :D_OUT], lhsT=ones_sb[:, :pbs],
                                 rhs=b2_sb[:], start=False, stop=True)
                otile = st_p.tile([P, D_OUT], dt.float32, tag="ot", name="otile")
                nc.vector.tensor_copy(otile[:pbs, :], ps2[:pbs, :D_OUT])
                nc.sync.dma_start(out_d.ap()[p0 : p0 + pbs, :], otile[:pbs, :])

    return nc


def make_in_maps(inputs, meta, per_core):
    x = np.asarray(inputs["x"], np.float32)[meta["perm"]]
    xp = x.astype(FP8).reshape(N_NODES // 2, 2 * D_IN)
    w1l = np.asarray(inputs["W1l"], np.float32)
    w1r = np.asarray(inputs["W1r"], np.float32)
    w2l = np.asarray(inputs["W2l"], np.float32)
    w2r = np.asarray(inputs["W2r"], np.float32)
    b1 = np.asarray(inputs["b1"], np.float32)
    b2 = np.asarray(inputs["b2"], np.float32)
    iota = np.tile(np.arange(P, dtype=np.float32)[None, :], (P, 1)).astype(BF16)
    in_maps = []
    for k in range(N_CORES):
        pc = per_core[k]
        im = {
            "xp": xp,
            "xT": np.ascontiguousarray(
                x[k * ROWS_PER : (k + 1) * ROWS_PER].T).astype(BF16),
            "invdeg": pc["invdeg"],
            "iota": iota,
            "w1lT": np.ascontiguousarray(w1l.T).astype(BF16),
            "w1rT": np.ascontiguousarray(w1r.T).astype(BF16),
            "w2lT": np.ascontiguousarray(w2l.T).astype(BF16),
            "w2rT": np.ascontiguousarray(w2r.T).astype(BF16),
            "b1r": b1[None, :].astype(BF16),
            "b2r": b2[None, :].astype(BF16),
        }
        for s in ("l1", "g1", "g2"):
            im["idx_" + s] = pc["idx_" + s]
            im["dstv_" + s] = pc["dstv_" + s]
        in_maps.append(im)
    return in_maps


_CACHE = {}


def _compile(meta):
    key = (meta["counts_key"], tuple(sorted(meta["labels"].items())))
    if key not in _CACHE:
        nc = bacc.Bacc("TRN2", target_bir_lowering=False, debug=False,
                       num_devices=N_CORES, num_swdge_queues=NQ)
        build_graph(nc, meta)
        nc.compile()
        _CACHE[key] = nc
    return _CACHE[key]


def assemble(res, meta):
    out = np.concatenate(
        [np.asarray(res.results[k]["out"]) for k in range(N_CORES)], axis=0
    ).astype(np.float32)
    unperm = np.empty_like(out)
    unperm[meta["perm"]] = out
    return unperm


def kernel(**inputs):
    edge_index = np.asarray(inputs["edge_index"])
    meta, per_core = preprocess(edge_index)
    nc = _compile(meta)
    in_maps = make_in_maps(inputs, meta, per_core)
    res = bass_utils.run_bass_kernel_spmd(
        nc, in_maps, core_ids=list(range(N_CORES))
    )
    return assemble(res, meta)
